# revision 1
# baseline (speedup 1.0000x reference)
"""DynaLoRALinear Trainium2 kernel (v2: bf16 operands).

Data-parallel over batch B across 8 NeuronCores (one sample per core).
Per core:
  - router:  logits = pooled @ (W_r @ gating_W).T  computed as a sharded
    partial (each core contracts over a 512-wide slice of D) + AllReduce.
  - gate weights from expert_scores ranks + module_prob>0.5 branch select.
  - base:    out = x_b @ W_base.T + b_base   (bf16 matmuls, fp32 PSUM accum)
  - lora:    t = x_b @ A_cat.T (fused into chunk-0 k-loop), then
             out += t @ (B_cat * gate).T
bf16 operands: 1 cyc/row on PE + fast-weight-load (FWL) halves LDWEIGHTS,
and DMA traffic halves vs f32r. absmax rel err ~1.5e-3 (gate 2e-2).
"""

import sys
import types

import numpy as np

B, L, D, E, R, NMOD = 8, 2048, 4096, 4, 8, 7
N_CORES = 8
DSH = D // N_CORES  # 512: per-core slice of D for the router shard
ER = E * R          # 32
KT = D // 128       # 32 k-tiles
MT = L // 128       # 16 m-tiles


def _to_bf16(a) -> np.ndarray:
    import ml_dtypes
    return np.ascontiguousarray(np.asarray(a, dtype=np.float32)
                                .astype(ml_dtypes.bfloat16))


def _install_profile_hook():
    """Make bass_utils' trace path importable (no-op if already present)."""
    try:
        import antenv.axon_hooks  # noqa: F401
        return
    except ImportError:
        pass
    try:
        import antenv
    except ImportError:
        return
    mod = types.ModuleType("antenv.axon_hooks")
    mod._hook = None
    mod.set_axon_ntff_profile_hook = lambda h: setattr(mod, "_hook", h)
    mod.get_axon_ntff_profile_hook = lambda: mod._hook
    sys.modules["antenv.axon_hooks"] = mod
    antenv.axon_hooks = mod
    try:
        from trn_agent_boot.trn_boot import _ntff_profile_via_ctypes
        hook = _ntff_profile_via_ctypes("/opt/axon/libaxon_pjrt.so")
        if hook is not None:
            mod.set_axon_ntff_profile_hook(hook)
    except Exception:
        pass


_PROGRAM_CACHE = {}


def _build_program(k: int, module_idx: int, has_bias: bool):
    import concourse.mybir as mybir
    import concourse.tile as tile
    from concourse import bacc
    from concourse.masks import make_identity

    f32 = mybir.dt.float32
    bf16 = mybir.dt.bfloat16
    alu = mybir.AluOpType
    act_fn = mybir.ActivationFunctionType

    k_lo = max(1, k // 2)

    nc = bacc.Bacc("TRN2", target_bir_lowering=False, debug=False,
                   num_devices=N_CORES)

    # --- DRAM I/O -------------------------------------------------------
    xT = nc.dram_tensor("xT", [D, L], bf16, kind="ExternalInput")
    WbT = nc.dram_tensor("WbT", [D, D], bf16, kind="ExternalInput")
    gw = nc.dram_tensor("gw", [D, DSH], bf16, kind="ExternalInput")
    # pre-packed on host to [128, ...] so the DMA is contiguous per
    # partition (the "(a p) m -> p a m" rearrange pattern costs thousands
    # of sub-64B descriptors)
    WrT = nc.dram_tensor("WrT", [128, KT * NMOD], bf16,
                         kind="ExternalInput")
    pooledT = nc.dram_tensor("pooledT", [128, 4 * B], f32,
                             kind="ExternalInput")
    scores_f = nc.dram_tensor("scores_f", [1, E * B], f32,
                              kind="ExternalInput")
    A_rhs = nc.dram_tensor("A_rhs", [128, KT * ER], bf16,
                           kind="ExternalInput")
    B_cat = nc.dram_tensor("B_cat", [ER, D], f32, kind="ExternalInput")
    b_row = nc.dram_tensor("b_row", [1, D], f32, kind="ExternalInput")
    msel = nc.dram_tensor("msel", [ER, E * B], f32, kind="ExternalInput")
    out = nc.dram_tensor("out", [L, D], f32, kind="ExternalOutput")

    with tile.TileContext(nc) as tc:
        with (
            tc.tile_pool(name="const", bufs=1) as const_pool,
            tc.tile_pool(name="gatep", bufs=1) as gate_pool,
            tc.tile_pool(name="rsb", bufs=1) as rsb,
            # W tiles: [128,512] bf16 = 1KB/partition. 2 full chunks
            # (2*32 tiles) resident -> next chunk always prefetches.
            tc.tile_pool(name="wpool", bufs=64) as wpool,
            tc.tile_pool(name="xpool", bufs=10) as xpool,
            tc.tile_pool(name="apool", bufs=1) as apool,
            tc.tile_pool(name="tpool", bufs=4) as tpool,
            tc.tile_pool(name="bpool", bufs=2) as bpool,
            tc.tile_pool(name="biasp", bufs=D // 512) as biasp,
            tc.tile_pool(name="epool", bufs=10) as epool,
            # chunk-0 base results parked in SBUF until gate arrives
            tc.tile_pool(name="e0pool", bufs=MT) as e0pool,
            tc.tile_pool(name="rgw", bufs=6) as rgw,
            tc.tile_pool(name="rdram", bufs=1, space="DRAM") as rdram,
        ):
            ident = const_pool.tile([128, 128], f32)
            make_identity(nc, ident)
            gate32 = gate_pool.tile([ER, 1], f32)

            bias_all = []
            if has_bias:
                for hh in range(D // 512):
                    bias_bc = biasp.tile([128, 512], f32, tag="biasbc",
                                         name=f"biasbc_{hh}")
                    nc.sync.dma_start(
                        bias_bc[0:1, :],
                        b_row[:, hh * 512:(hh + 1) * 512])
                    nc.gpsimd.partition_broadcast(bias_bc[:],
                                                  bias_bc[0:1, :])
                    bias_all.append(bias_bc)

            # ====== router inputs (small DMAs only; the router matmuls
            # are interleaved into chunk-0 group-0's k-loop below so the
            # PE starts on base work almost immediately) ================
            wr_sb = rsb.tile([128, KT, NMOD], bf16)
            pt_sb = rsb.tile([128, 4, B], f32)
            msel_sb = rsb.tile([ER, E * B], f32)
            sc = rsb.tile([1, E * B], f32)
            a_sb = apool.tile([128, KT, ER], bf16)
            lg_sb = rsb.tile([NMOD, B], f32)

            def emit_small_dmas():
                # emitted just after chunk-0 kt0's x/W/gw DMAs so the
                # PE's first base matmul isn't queued behind these
                nc.sync.dma_start(a_sb[:], A_rhs[:])
                nc.sync.dma_start(wr_sb[:], WrT[:])
                nc.sync.dma_start(pt_sb[:], pooledT[:])
                nc.sync.dma_start(msel_sb[:], msel[:])
                nc.sync.dma_start(sc[:], scores_f[:])

            def emit_router_tail(mps, wc_ps):
                """Router part-A tail: logits partial + AllReduce kick."""
                wc_sb = rsb.tile([NMOD, DSH], f32)
                nc.vector.tensor_copy(wc_sb[:], wc_ps[:])
                wct = rsb.tile([128, 4 * NMOD], f32)
                for j in range(4):
                    tp = mps.tile([128, NMOD], f32, tag="ps",
                                  name=f"tp_{j}")
                    nc.tensor.transpose(
                        tp[:], wc_sb[:, j * 128:(j + 1) * 128],
                        ident[0:NMOD, 0:NMOD])
                    nc.vector.tensor_copy(
                        wct[:, j * NMOD:(j + 1) * NMOD], tp[:])
                lg_ps = mps.tile([NMOD, B], f32, tag="ps", name="lg_ps")
                for j in range(4):
                    nc.tensor.matmul(lg_ps[:],
                                     wct[:, j * NMOD:(j + 1) * NMOD],
                                     pt_sb[:, j, :],
                                     start=(j == 0), stop=(j == 3))
                lp_sb = rsb.tile([NMOD, B], f32)
                nc.vector.tensor_copy(lp_sb[:], lg_ps[:])
                cc_in = rdram.tile([NMOD, B], f32)
                cc_out = rdram.tile([NMOD, B], f32)
                nc.gpsimd.dma_start(cc_in[:], lp_sb[:])
                nc.gpsimd.collective_compute(
                    "AllReduce", alu.add,
                    replica_groups=[list(range(N_CORES))],
                    ins=[cc_in.opt()], outs=[cc_out.opt()])
                nc.gpsimd.dma_start(lg_sb[:], cc_out[:])

                # collective-independent: expert ranks from scores
                rank = rsb.tile([1, E * B], f32)
                nc.vector.memset(rank[:], 0.0)
                tmp = rsb.tile([1, B], f32)
                for e in range(E):
                    re = rank[:, e * B:(e + 1) * B]
                    se = sc[:, e * B:(e + 1) * B]
                    for e2 in range(E):
                        if e2 == e:
                            continue
                        s2 = sc[:, e2 * B:(e2 + 1) * B]
                        nc.vector.tensor_tensor(tmp[:], s2, se, op=alu.is_gt)
                        nc.vector.tensor_add(re, re, tmp[:])
                        if e2 < e:
                            nc.vector.tensor_tensor(tmp[:], s2, se,
                                                    op=alu.is_equal)
                            nc.vector.tensor_add(re, re, tmp[:])
                w_hi = rsb.tile([1, E * B], f32)
                nc.vector.tensor_scalar(w_hi[:], rank[:], float(k),
                                        1.0 / float(k),
                                        op0=alu.is_lt, op1=alu.mult)
                w_lo = rsb.tile([1, E * B], f32)
                nc.vector.tensor_scalar(w_lo[:], rank[:], float(k_lo),
                                        1.0 / float(k_lo),
                                        op0=alu.is_lt, op1=alu.mult)
                diff = rsb.tile([1, E * B], f32)
                nc.vector.tensor_sub(diff[:], w_hi[:], w_lo[:])
                return w_lo, diff

            # ====== router part B (emitted mid-chunk-0 below) ==========
            def emit_part_b(mps, w_lo, diff):
                ltp = mps.tile([B, NMOD], f32, tag="ps", name="ltp")
                nc.tensor.transpose(ltp[:], lg_sb[:], ident[0:NMOD, 0:NMOD])
                lt = rsb.tile([B, NMOD], f32)
                nc.vector.tensor_copy(lt[:], ltp[:])
                mx = rsb.tile([B, 1], f32)
                nc.vector.tensor_reduce(out=mx[:], in_=lt[:], op=alu.max,
                                        axis=mybir.AxisListType.X)
                mxn = rsb.tile([B, 1], f32)
                nc.vector.tensor_scalar_mul(mxn[:], mx[:], -1.0)
                ex = rsb.tile([B, NMOD], f32)
                nc.scalar.activation(ex[:], lt[:], act_fn.Exp, bias=mxn[:])
                sm = rsb.tile([B, 1], f32)
                nc.vector.tensor_reduce(out=sm[:], in_=ex[:], op=alu.add,
                                        axis=mybir.AxisListType.X)
                rs = rsb.tile([B, 1], f32)
                nc.vector.reciprocal(rs[:], sm[:])
                p0 = rsb.tile([B, 1], f32)
                nc.vector.tensor_mul(
                    p0[:], ex[:, module_idx:module_idx + 1], rs[:])
                hi = rsb.tile([B, 1], f32)
                nc.vector.tensor_single_scalar(hi[:], p0[:], 0.5, alu.is_gt)
                hp = mps.tile([1, B], f32, tag="ps", name="hp")
                nc.tensor.transpose(hp[:], hi[:], ident[0:B, 0:B])
                hi_row = rsb.tile([1, B], f32)
                nc.vector.tensor_copy(hi_row[:], hp[:])
                gate = rsb.tile([1, E * B], f32)
                for e in range(E):
                    nc.vector.tensor_mul(gate[:, e * B:(e + 1) * B],
                                         diff[:, e * B:(e + 1) * B],
                                         hi_row[:])
                nc.vector.tensor_add(gate[:], gate[:], w_lo[:])
                gateb = rsb.tile([ER, E * B], f32)
                nc.gpsimd.partition_broadcast(gateb[:], gate[:])
                g32m = rsb.tile([ER, E * B], f32)
                nc.vector.tensor_tensor(g32m[:], gateb[:], msel_sb[:],
                                        op=alu.mult)
                nc.vector.tensor_reduce(out=gate32[:], in_=g32m[:],
                                        op=alu.add,
                                        axis=mybir.AxisListType.X)

            # ============== main: base + lora ==========================
            with (
                tc.tile_pool(name="mps", bufs=8, space="PSUM") as mps,
            ):
                # 8 chunks of 512 out-cols. Chunk 0 runs unsplit K=128
                # matmuls (its PSUM budget also carries ps_t + wc_ps);
                # chunks 1-7 split each matmul into two K=64 row-tiles at
                # tile_position (0,0)/(64,0) into separate lo/hi banks —
                # concurrent row-groups hide the stationary-swap drain
                # (~50ns/MM, measured 260.6 -> 217.0 ns per k-tile).
                CHUNKS = list(range(0, D, 512))
                tT_tiles = [None] * (MT // 4)
                e0_tiles = [None] * MT
                w_lo = diff = None
                GS = 4
                NG = MT // GS

                def store_out(ev, m, col0):
                    # stores ride the ACT HWDGE ring so they never
                    # head-block x/W loads on the sync ring
                    nc.scalar.dma_start(
                        out[m * 128:(m + 1) * 128, col0:col0 + 512], ev[:])

                def evict0(ps, m):
                    """Chunk 0: park base-only PSUM in SBUF."""
                    ev = e0pool.tile([128, 512], f32, tag="e0",
                                     name=f"e0_{m}")
                    e0_tiles[m] = ev
                    if has_bias:
                        nc.vector.tensor_add(ev[:], ps[:], bias_all[0][:])
                    elif m % 2 == 1:
                        nc.scalar.activation(ev[:], ps[:], act_fn.Copy)
                    else:
                        nc.vector.tensor_copy(ev[:], ps[:])

                def evict_split(ps_lo, ps_hi, m, col0, c):
                    """lo+hi partial sums -> SBUF -> DRAM."""
                    tmp = epool.tile([128, 512], f32, tag="etmp",
                                     name=f"etmp_{c}_{m}")
                    nc.scalar.activation(tmp[:], ps_hi[:], act_fn.Copy)
                    ev = epool.tile([128, 512], f32, tag="ev",
                                    name=f"ev_{c}_{m}")
                    if has_bias:
                        # ev = ps_lo + tmp, then += bias
                        nc.vector.tensor_add(ev[:], ps_lo[:], tmp[:])
                        nc.vector.tensor_add(ev[:], ev[:],
                                             bias_all[col0 // 512][:])
                    else:
                        nc.vector.tensor_add(ev[:], ps_lo[:], tmp[:])
                    store_out(ev, m, col0)

                prefetched = None
                for c, col0 in enumerate(CHUNKS):
                    wtiles = [None] * KT if c == 0 else prefetched
                    if c + 1 < len(CHUNKS):
                        prefetched = [None] * KT
                        n_pref = KT
                    else:
                        n_pref = 0

                    def prefetch_w(kt_n, c=c, col_next=(
                            CHUNKS[c + 1] if c + 1 < len(CHUNKS) else 0)):
                        wt = wpool.tile([128, 512], bf16, tag="w",
                                        name=f"w_{c + 1}_{kt_n}")
                        nc.sync.dma_start(
                            wt[:],
                            WbT[kt_n * 128:(kt_n + 1) * 128,
                                col_next:col_next + 512])
                        prefetched[kt_n] = wt

                    # B slice scaled by this core's gate. Chunk 0's scale
                    # must wait for part B (emitted below at mg==2).
                    b_stg = bpool.tile([ER, 512], f32, tag="bstg",
                                       name=f"bstg_{c}")
                    nc.sync.dma_start(b_stg[:],
                                      B_cat[:, col0:col0 + 512])
                    if c == 0:
                        bstg0 = b_stg
                        bh = None
                    else:
                        bh = bpool.tile([ER, 512], bf16, tag="bscl",
                                        name=f"bscl_{c}")
                        nc.vector.tensor_scalar_mul(bh[:], b_stg[:],
                                                    gate32[:, 0:1])

                    for mg in range(NG):
                        if c == 0:
                            pss = [mps.tile([128, 512], f32, tag="ps",
                                            name=f"ps_{c}_{mg}_{mi}")
                                   for mi in range(GS)]
                            ps_t = mps.tile([ER, 512], f32, tag="ps",
                                            name=f"pst_{mg}")
                            if mg == 0:
                                wc_ps = mps.tile([NMOD, DSH], f32,
                                                 tag="ps", name="wc_ps")
                        else:
                            pss = [(mps.tile([128, 512], f32, tag="ps",
                                             name=f"pl_{c}_{mg}_{mi}"),
                                    mps.tile([128, 512], f32, tag="ps",
                                             name=f"ph_{c}_{mg}_{mi}"))
                                   for mi in range(GS)]
                        for kt in range(KT):
                            xs = xpool.tile([128, GS * 128], bf16, tag="x",
                                            name=f"x_{c}_{mg}_{kt}")
                            nc.sync.dma_start(
                                xs[:],
                                xT[kt * 128:(kt + 1) * 128,
                                   mg * GS * 128:(mg + 1) * GS * 128])
                            if mg >= 1:
                                # 32 tiles over 96 k-slots on the sync ring
                                slot = (mg - 1) * KT + kt
                                if slot % 3 == 0 and slot // 3 < n_pref:
                                    prefetch_w(slot // 3)
                            if c == 0 and mg == 0:
                                # stream chunk-0 W and router gating_W
                                # tiles just-in-time alongside x
                                wt = wpool.tile([128, 512], bf16, tag="w",
                                                name=f"w_0_{kt}")
                                nc.sync.dma_start(
                                    wt[:],
                                    WbT[kt * 128:(kt + 1) * 128, 0:512])
                                wtiles[kt] = wt
                                if kt == 0:
                                    emit_small_dmas()
                                gwt = rgw.tile([128, DSH], bf16, tag="gwt",
                                               name=f"gwt_{kt}")
                                nc.sync.dma_start(
                                    gwt[:], gw[kt * 128:(kt + 1) * 128, :])
                            last = kt == KT - 1
                            for mi in range(GS):
                                m = mg * GS + mi
                                xsl = xs[:, mi * 128:(mi + 1) * 128]
                                if c == 0:
                                    nc.tensor.matmul(
                                        pss[mi][:], xsl, wtiles[kt][:],
                                        start=(kt == 0), stop=False)
                                    if last:
                                        evict0(pss[mi], m)
                                else:
                                    lo, hi = pss[mi]
                                    nc.tensor.matmul(
                                        lo[:], xsl[0:64, :],
                                        wtiles[kt][0:64, :],
                                        start=(kt == 0), stop=False,
                                        tile_position=(0, 0))
                                    nc.tensor.matmul(
                                        hi[:], xsl[64:128, :],
                                        wtiles[kt][64:128, :],
                                        start=(kt == 0), stop=last,
                                        tile_position=(64, 0))
                                    if last:
                                        tsl = tT_tiles[m // 4][
                                            :, (m % 4) * 128:
                                            (m % 4) * 128 + 128]
                                        nc.tensor.matmul(
                                            lo[:], tsl, bh[:],
                                            start=False, stop=True,
                                            tile_position=(0, 0))
                                        evict_split(lo, hi, m, col0, c)
                            if c == 0:
                                nc.tensor.matmul(
                                    ps_t[:], a_sb[:, kt, :], xs[:],
                                    start=(kt == 0), stop=(kt == KT - 1))
                            if c == 0 and mg == 0:
                                nc.tensor.matmul(wc_ps[:], wr_sb[:, kt, :],
                                                 gwt[:],
                                                 start=(kt == 0),
                                                 stop=(kt == KT - 1))
                        if c == 0:
                            tT = tpool.tile([ER, 512], bf16, tag="tT",
                                            name=f"tT_{mg}")
                            nc.vector.tensor_copy(tT[:], ps_t[:])
                            tT_tiles[mg] = tT
                        if c == 0 and mg == 0:
                            # router logits partial + AllReduce kick-off
                            w_lo, diff = emit_router_tail(mps, wc_ps)
                        if c == 0 and mg == 2:
                            # gate computation (the AllReduce result has
                            # had ~2 group-times to land)
                            emit_part_b(mps, w_lo, diff)
                            b_scl0 = bpool.tile([ER, 512], bf16,
                                                tag="bscl", name="bscl_0_0")
                            nc.vector.tensor_scalar_mul(
                                b_scl0[:], bstg0[:], gate32[:, 0:1])
                    if c == 0:
                        # deferred chunk-0 lora: combine into fresh psum,
                        # add the parked base tile, store.
                        for m in range(MT):
                            tsl = tT_tiles[m // 4][:, (m % 4) * 128:
                                                   (m % 4) * 128 + 128]
                            lp = mps.tile([128, 512], f32, tag="ps",
                                          name=f"lp_{m}")
                            nc.tensor.matmul(lp[:], tsl, b_scl0[:],
                                             start=True, stop=True)
                            # in-place: e0 already holds the base part, so
                            # no fresh evict tile (avoids pool-recycle
                            # waits on store receipts)
                            nc.vector.tensor_add(e0_tiles[m][:], lp[:],
                                                 e0_tiles[m][:])
                            nc.scalar.dma_start(
                                out[m * 128:(m + 1) * 128, 0:512],
                                e0_tiles[m][:])

    nc.compile()
    return nc


def kernel(**inputs) -> np.ndarray:
    _install_profile_hook()

    x = np.asarray(inputs["x"], dtype=np.float32)
    expert_scores = np.asarray(inputs["expert_scores"], dtype=np.float32)
    W_base = np.asarray(inputs["W_base"], dtype=np.float32)
    b_base = np.asarray(inputs["b_base"], dtype=np.float32)
    gating_W = np.asarray(inputs["gating_W"], dtype=np.float32)
    W_r = np.asarray(inputs["W_r"], dtype=np.float32)
    lora_A = np.asarray(inputs["lora_A"], dtype=np.float32)
    lora_B = np.asarray(inputs["lora_B"], dtype=np.float32)
    module_idx = int(np.asarray(inputs["module_idx"]))
    k = int(np.asarray(inputs["k"]))

    has_bias = bool(np.any(b_base != 0.0))
    key = (k, module_idx, has_bias)
    if key not in _PROGRAM_CACHE:
        _PROGRAM_CACHE[key] = _build_program(k, module_idx, has_bias)
    nc = _PROGRAM_CACHE[key]

    # --- host-side layout prep (transposes/slices/bf16 rounding) --------
    WbT_np = _to_bf16(W_base.T)                          # [D, D]
    # packed [128, KT*NMOD]: row p holds W_r.T[kt*128+p, :] for each kt
    WrT_np = _to_bf16(W_r.T.reshape(KT, 128, NMOD)
                      .transpose(1, 0, 2).reshape(128, KT * NMOD))
    A_np = _to_bf16(lora_A.reshape(ER, D).T.reshape(KT, 128, ER)
                    .transpose(1, 0, 2).reshape(128, KT * ER))
    B_np = np.ascontiguousarray(
        lora_B.transpose(0, 2, 1).reshape(ER, D))        # [ER, D] fp32
    scores_f_np = np.ascontiguousarray(
        expert_scores.T.reshape(1, E * B))               # [1, E*B]
    b_row_np = b_base.reshape(1, D)
    pooled = x[:, -1, :]                                 # [B, D]

    in_maps = []
    for c in range(N_CORES):
        msel_np = np.zeros((ER, E, B), dtype=np.float32)
        for p in range(ER):
            msel_np[p, p // R, c] = 1.0
        msel_np = msel_np.reshape(ER, E * B)
        in_maps.append({
            "xT": _to_bf16(x[c].T),
            "WbT": WbT_np,
            "gw": _to_bf16(gating_W[:, c * DSH:(c + 1) * DSH]),
            "WrT": WrT_np,
            "pooledT": np.ascontiguousarray(
                pooled[:, c * DSH:(c + 1) * DSH].T.reshape(4, 128, B)
                .transpose(1, 0, 2).reshape(128, 4 * B)),
            "scores_f": scores_f_np,
            "A_rhs": A_np,
            "B_cat": B_np,
            "b_row": b_row_np,
            "msel": msel_np,
        })

    from concourse.bass_utils import run_bass_kernel_spmd
    res = run_bass_kernel_spmd(nc, in_maps, core_ids=list(range(N_CORES)))
    return np.stack([res.results[c]["out"] for c in range(N_CORES)], axis=0)


if __name__ == "__main__":
    rng = np.random.default_rng(0)
    demo = {
        "x": (rng.standard_normal((B, L, D)) * 0.02).astype(np.float32),
        "expert_scores": rng.random((B, E), dtype=np.float32),
        "W_base": (rng.standard_normal((D, D)) * 0.02).astype(np.float32),
        "b_base": np.zeros(D, np.float32),
        "gating_W": (rng.standard_normal((D, D)) * 0.02).astype(np.float32),
        "W_r": (rng.standard_normal((NMOD, D)) * 0.02).astype(np.float32),
        "lora_A": (rng.standard_normal((E, R, D)) * 0.02).astype(np.float32),
        "lora_B": (rng.standard_normal((E, D, R)) * 0.02).astype(np.float32),
        "module_idx": 0,
        "k": 2,
    }
    y = kernel(**demo)
    print("out", y.shape, y.dtype, float(np.abs(y).max()))

